# revision 51
# baseline (speedup 1.0000x reference)
"""LocallyConnected1d (B=32, C=32, L=4096, K=7, stride=1) Trainium2 Bass kernel.

Strategy (hardcoded for this problem):
  - Shard L_out=4090 across 8 cores (sequence parallel), 512 positions/core
    (padded; core 7 carries 6 zero-padded positions). Everything moves in
    bf16 (inputs quantized on host; psum accumulates f32; output staged bf16
    and upcast on host). All weight bytes are read from HBM exactly once.
  - Positions are processed in blocks of 4 (m = l//4, c = l%4). Per block,
    ONE pair of stationary loads covers all 4 positions:
      S1[m] = w[o, i, 4m+c, kk]    [(kk,i) 128 x (c,o) 128]   taps 0..3
      S2[m] = w[o, i, 4m+c, 4+kk]  [(kk,i)  96 x (c,o) 128]   taps 4..6
    The moving operand packs 4 shifted x windows as 128 columns:
      xm[(kk,i), (m, c', b)] = x[b, i, 4m + c' + kk]
    psum[(c,o), (c',b)] += S1[m]^T xm[:, m] ;  += S2[m]^T xm[:96, m+1]
    Only the c == c' diagonal 32x32 blocks are valid; the drain extracts
    them. This costs 4x moving-stream redundancy but cuts PE stationary
    loads 4x (the real-HW bottleneck: loads are ~1 row/cycle and weights
    here are unshared, so every block needs fresh stationaries).
  - x is uploaded tap-band-replicated from HBM (4x bytes, ~4.5 MB/core):
    measured faster than building the bands on-chip (real engine copies
    are ~3x the cost model, and the x->bands->matmul chain serializes).
  - Uniform pipeline of 8 16-block groups. Per group: one x piece (17
    blocks incl. tap seam), one w1 + one w2 piece, one psum tile
    [128, 2048] (bufs=2 - ping-pong over the full 16 KB PSUM), 4 drains
    (Act engine; the final group splits DVE/Act), output staged bf16 per
    group-pair, 4 output DMAs at the end of SP's queue.
"""

import sys

if "/opt/trn_rl_repo" not in sys.path:
    sys.path.insert(0, "/opt/trn_rl_repo")

import ml_dtypes
import numpy as np

import bass_rust
from concourse import bass, mybir, tile
from concourse.bass_utils import run_bass_kernel_spmd

# Problem constants (hardcoded; must match the grading reference).
B = 32          # batch
IC = 32         # in channels
L = 4096        # input length
OC = 32         # out channels
K = 7           # kernel taps
L_OUT = 4090    # (L - (K-1)) // 1

NCORES = 8
LP = 512        # positions per core (padded: 8*512 = 4096 >= 4090)
NBLK = LP // 4  # 128 four-position blocks per core

PSBLK = 16            # blocks per psum tile / pipeline group
XGC = (PSBLK + 1) * 128  # 2176: x cols per group (16 blocks + tap seam)
XDCOLS = (NBLK // PSBLK) * XGC  # 17408 x dram cols (group-major, replicated)
WCOLS = NBLK * 128    # 16384 weight dram cols (m, c, o)
PSCOLS = PSBLK * 128  # 2048 f32 cols = 8 KB/partition (half of PSUM)
OCOLS = NBLK * B      # 4096 out dram cols (m, b)

DRAIN_MID = "ssss"   # drain engine per column group (mid groups)
DRAIN_LAST = "vsvs"  # final group (DVE is idle by then)
F32 = mybir.dt.float32
BF16 = mybir.dt.bfloat16
NP_BF16 = ml_dtypes.bfloat16

_CACHE = {}


def _ap(t_ap, offset, dims):
    """Build a raw access pattern on the tensor behind an AP."""
    return bass_rust.AP(t_ap.tensor, int(offset), [[int(s), int(n)] for s, n in dims])


def _emit(reps=None):
    """Build the (identical-per-core) single-core program.

    reps: if set, wrap the whole body (DMAs included) in a hardware loop that
    executes it `reps` times -- used only for wall-clock timing calibration.
    """
    import contextlib

    nc = bass.Bass()
    x_d = nc.dram_tensor("xb", [128, XDCOLS], BF16, kind="ExternalInput")
    w1_d = nc.dram_tensor("w1", [128, WCOLS], BF16, kind="ExternalInput")
    w2_d = nc.dram_tensor("w2", [96, WCOLS], BF16, kind="ExternalInput")
    o_d = nc.dram_tensor("out", [128, OCOLS], BF16, kind="ExternalOutput")

    with tile.TileContext(nc) as tc:
        with (
            tc.tile_pool(name="xpool", bufs=4) as xpool,
            tc.tile_pool(name="w1pool", bufs=4) as w1pool,
            tc.tile_pool(name="w2pool", bufs=4) as w2pool,
            tc.tile_pool(name="opool", bufs=4) as opool,
            tc.tile_pool(name="psum", bufs=2, space=bass.MemorySpace.PSUM) as psum,
        ):
            loop = (
                tc.For_i(0, reps, 1, hint_engines=(mybir.EngineType.PE,))
                if reps is not None else contextlib.nullcontext()
            )
            with loop:
                _emit_body(nc, x_d, w1_d, w2_d, o_d,
                           xpool, w1pool, w2pool, opool, psum)
    _split_matmul_waits(nc)
    return nc


def _emit_body(nc, x_d, w1_d, w2_d, o_d, xpool, w1pool, w2pool, opool, psum):
    NG = NBLK // PSBLK  # 8 uniform 16-block groups
    OCC = 2 * PSBLK * B  # out staging covers two groups
    eng_map = {"v": nc.vector.tensor_copy, "s": nc.scalar.copy,
               "p": nc.gpsimd.tensor_copy}
    ocs = [None] * (NG // 2)
    for g in range(NG):
        # per-group x piece: 17 blocks (16 + seam block for taps 4..6)
        xt = xpool.tile([128, XGC], BF16, tag="xc", name=f"xg{g}")
        xc = xt[:]
        nc.sync.dma_start(
            _ap(xc, 0, [[XGC, 128], [1, XGC]]),
            _ap(x_d[:], g * XGC, [[XDCOLS, 128], [1, XGC]]),
        )
        WH = PSBLK * 128
        w1t = w1pool.tile([128, WH], BF16, tag="w1c", name=f"w1g{g}")
        w1c = w1t[:]
        nc.sync.dma_start(
            _ap(w1c, 0, [[WH, 128], [1, WH]]),
            _ap(w1_d[:], g * WH, [[WCOLS, 128], [1, WH]]),
        )
        w2t = w2pool.tile([128, WH], BF16, tag="w2c", name=f"w2g{g}")
        w2c = w2t[:]
        nc.sync.dma_start(
            _ap(w2c, 0, [[WH, 96], [1, WH]]),
            _ap(w2_d[:], g * WH, [[WCOLS, 96], [1, WH]]),
        )
        if g % 2 == 0:
            ot = opool.tile([128, OCC], BF16, tag="oc", name=f"oc{g//2}")
            ocs[g // 2] = ot[:]
        oc = ocs[g // 2]

        pgt = psum.tile([128, PSCOLS], F32, tag="ps", name=f"ps{g}")
        pg = pgt[:]
        for mb in range(PSBLK):
            out_ap = _ap(pg, mb * 128, [[PSCOLS, 128], [1, 128]])
            # taps 0..3: contraction over 128 partitions (kk, i)
            nc.tensor.matmul(
                out_ap,
                _ap(w1c, mb * 128, [[WH, 128], [1, 128]]),
                _ap(xc, mb * 128, [[XGC, 128], [1, 128]]),
                start=True, stop=False,
                tile_position=(0, 0), skip_group_check=True,
            )
            # taps 4..6: contraction over 96 partitions, x block m+1
            nc.tensor.matmul(
                out_ap,
                _ap(w2c, mb * 128, [[WH, 96], [1, 128]]),
                _ap(xc, (mb + 1) * 128, [[XGC, 96], [1, 128]]),
                start=False, stop=True,
                tile_position=(0, 0), skip_group_check=True,
            )
        # drain the c==c' diagonal 32x32 blocks of the 16 finished
        # block-outputs; one instruction per column group c (Act mostly;
        # DVE helps on the final group where nothing else needs it)
        pat = DRAIN_LAST if g == NG - 1 else DRAIN_MID
        for c in range(4):
            eng_map[pat[c]](
                _ap(oc, (32 * c) * OCC + (g % 2) * PSBLK * B,
                    [[OCC, 32], [B, PSBLK], [1, B]]),
                _ap(pg, (32 * c) * PSCOLS + 32 * c,
                    [[PSCOLS, 32], [128, PSBLK], [1, B]]),
            )
    # output DMAs at the end of SP's queue: they wait on drains without
    # blocking any input DMA issue
    for jo in range(NG // 2):
        nc.sync.dma_start(
            _ap(o_d[:], jo * OCC, [[OCOLS, 128], [1, OCC]]),
            _ap(ocs[jo], 0, [[OCC, 128], [1, OCC]]),
        )


def _split_matmul_waits(nc):
    """This walrus build allows at most one sync wait per instruction.
    Relocate each multi-wait instruction's waits onto a chain of single-wait
    NoOps inserted just before it on the same engine -- program order makes
    this semantically identical."""
    for f in nc.m.functions:
        for bb in f.blocks:
            insts = list(bb.instructions)
            out = []
            changed = False
            for ins in insts:
                si = ins.sync_info
                if (si is not None and si.on_wait
                        and len(si.on_wait) >= 2):
                    for w in si.on_wait:
                        nop = mybir.InstNoOp(
                            name=nc.get_next_instruction_name(),
                            ins=[], outs=[],
                            sync_info=mybir.SyncInfo(
                                on_wait=[w], on_update=[]),
                            bass_nofuse=True,
                            engine=ins.engine,
                        )
                        nc.inst_map[nop.name] = nop
                        out.append(nop)
                    ins.sync_info = mybir.SyncInfo(
                        on_wait=[], on_update=list(si.on_update))
                    changed = True
                out.append(ins)
            if changed:
                bb.instructions = out


def _get_nc():
    if "nc" not in _CACHE:
        _CACHE["nc"] = _emit()
    return _CACHE["nc"]


def _shard_inputs(x, weight):
    """Pre-permute full inputs into the per-core bf16 kernel layouts."""
    x = np.asarray(x, dtype=np.float32)
    weight = np.asarray(weight, dtype=np.float32)
    # x positions needed per core: l0 .. l0 + 4*NBLK + 6 (tap seam)
    xpad = np.zeros((B, IC, NCORES * LP + 544), dtype=np.float32)
    xpad[:, :, :L] = x
    wpad = np.zeros((OC, IC, NCORES * LP, K), dtype=np.float32)
    wpad[:, :, :L_OUT, :] = weight

    NG = NBLK // PSBLK
    NBJ = PSBLK + 1  # blocks per x group piece incl. seam
    in_maps = []
    for m in range(NCORES):
        l0 = m * LP
        # xb[(kk,i), (group g: m_loc, c, b)] = x[b, i, 4*(g*PSBLK+m_loc)+c+kk]
        xb = np.empty((4, IC, NG, NBJ * 4 * B), dtype=np.float32)
        for g in range(NG):
            p0 = l0 + 4 * g * PSBLK
            for kk in range(4):
                # (B, IC, 4*NBJ) -> (IC, NBJ*4, B) -> (IC, cols)
                sl = xpad[:, :, p0 + kk : p0 + kk + 4 * NBJ]
                xb[kk, :, g] = sl.transpose(1, 2, 0).reshape(IC, -1)
        xb = xb.transpose(0, 1, 2, 3).reshape(128, XDCOLS)
        ws = wpad[:, :, l0 : l0 + LP, :]        # (OC, IC, LP, K)
        # w[o, i, 4m + c, k] -> [k, i, (m, c, o)]
        wt = ws.reshape(OC, IC, NBLK, 4, K).transpose(4, 1, 2, 3, 0)
        in_maps.append({
            "xb": np.ascontiguousarray(xb).astype(NP_BF16),
            "w1": np.ascontiguousarray(wt[0:4]).reshape(128, WCOLS).astype(NP_BF16),
            "w2": np.ascontiguousarray(wt[4:7]).reshape(96, WCOLS).astype(NP_BF16),
        })
    return in_maps


def _unshard_output(res):
    """res: list of per-core {"out": (128, OCOLS)} -> full (B, OC, L_OUT)."""
    out = np.empty((B, OC, NCORES * LP), dtype=np.float32)
    for m in range(NCORES):
        # partition (c, o), col (m_blk, b)
        arr = res[m]["out"].astype(np.float32).reshape(4, OC, NBLK, B)
        # -> (b, o, m_blk, c) -> (B, OC, LP)
        out[:, :, m * LP : (m + 1) * LP] = (
            arr.transpose(3, 1, 2, 0).reshape(B, OC, LP)
        )
    return np.ascontiguousarray(out[:, :, :L_OUT])


def kernel(x, weight):
    nc = _get_nc()
    in_maps = _shard_inputs(x, weight)
    res = run_bass_kernel_spmd(nc, in_maps, list(range(NCORES))).results
    return _unshard_output(res)
